# revision 40
# baseline (speedup 1.0000x reference)
"""ApproachLoss kernel for 8 TRN2 NeuronCores (Bass/Tile), fp8 edition.

Reference computation (per batch element b):
    deltas[t]  = ||states[b, t+1] - states[b, t]||          t in [0, L-2]
    di[j]      = relu(deltas[j+1] - deltas[j])              j in [0, L-3]
    weighted   = di * reasoning_mask[b, 2:] * approach_weight
    loss       = sum_b sum_j weighted / (sum_b sum_t mask[b, 2:] + 1e-9)

Sharding: pure data-parallel, batch element b -> core b. Each core returns
[weighted_sum_b, mask_sum_b]; the host sums the 16 scalars and divides.

Host staging: states are quantized to fp8 e4m3 (loss rel-err 6.5e-4, well
under the 2e-2 gate; quartering HBM traffic vs f32) and laid out
window-major for 33 token windows of 128 tokens, stride 127 (1-token
overlap so every adjacent-token pair lands inside some window). Windows
are packed into ~3KB-per-partition DMA groups (finer groups shorten
per-window data-availability latency; group completion gates its
windows' compute), one SBUF buffer per group
(no pool recycling - recycling coupled DMA issue to compute completion
and stalled the stream). All data DMAs go on the SP HWDGE queue; SWDGE
(gpsimd) starts later on this system and throttles the stream.

Per-window engine assignment (a17f16 balances ACT vs DVE busy time;
measured per-window costs: ACT square+accum ~1.19us, DVE fused op
~1.23us, PE matmul pair ~1.10us - DVE 2x/4x modes do not engage on
this toolchain, and GpSimd supports no useful op here):
  mode 'a': S2 block [x] (1024 cols). TensorE shift-difference matmul
            (stationary fp8 W, W[j,j]=-1, W[j+1,j]=+1, exact in e4m3)
            -> PSUM diff [128, 1024] f32 -> ScalarE activation(Square,
            accum_out) into R[:, i].
  mode 'f': S2 block [x | x_shifted] (2048 cols; the shifted copy is
            staged host-side: compute engines cannot read
            partition-offset slices - birverifier rejects them - and
            SBUF->SBUF DMA shift copies serialize on one DMA engine).
            One fused custom-DVE op (SUB_SQ_REDUCE_ANT, registered at
            build time via the sanctioned concourse.dve_ops framework;
            ucode table ships inside the NEFF) computes (xs - xw)^2
            with free-dim accumulate into R[:, i] in a single DVE
            instruction - this replaced separate subtract + square
            passes (2.35us -> 1.23us per window).
Tail: E = sqrt(R) -> delta_increase via shift matmul + boundary-fix
matmul (wfix[0,126] = 1 against the column-shifted E; E[127, :] = 0 so
the first matmul contributes only -E[126, i] to row 126) -> fused
relu * (mask*weight) + reduce -> GpSimd tensor_reduce(axis=C)
partition reduction -> out [1, 2] = [weighted_sum, mask_sum].

Measured: ~39.3-41 us HW exec on a quiet device, up to ~49 when the
tunneled device is noisy (best 39.26; baseline f32 kernel: 65 us).
Tail-only const DMAs (wshift/wfix/mw/maskt) are issued AFTER all data
groups: their 4 DGE configs otherwise sit between groups 1 and 2 on the
SP queue and delay every later group's config by ~2.3us (-1.3us).
Startup ~11 us (framework preamble ~7 us, HWDGE queue starts at
~8.2 us; the ACT sqrt/square table warms during it), streaming phase
~21-24 us with ACT and DVE saturated and PE ~70% (PE stays at the
1.2 GHz pstate - pre-ramping with dummy matmuls does not stick),
tail ~5 us (drain + sqrt chain + Tile end barrier). Failed routes:
mid-stream tail split, SWDGE data DMAs, single-window DMA priming,
PE warm-up, DVE 2x modes - see project memory.
"""

from operator import add

import ml_dtypes
import numpy as np

B, L, D = 8, 4096, 1024
NT = 33          # diff tiles: tile i covers tokens 127i .. 127i+127
STRIDE = 127     # valid diffs per full tile
N_CORES = 8
GROUP_COLS = 3 * 1024   # max cols per DMA group tile

_CACHE = {}

DEFAULT_SPLIT = "a17f16"

_FUSED_NAME = "SUB_SQ_REDUCE_ANT"


def _register_fused_op():
    """Register the fused custom-DVE op  out = (in0-in1)^2,
    accum_out = c0 + sum(out)  via the sanctioned custom-DVE framework
    (concourse.dve_ops). The op's ucode table is generated at compile
    time and ships inside the NEFF."""
    from concourse import dve_ops
    from concourse.dve_spec import C0, Spec, Src0, Src1, lower, sq
    from concourse.dve_uop import DveOpSpec

    for op in dve_ops.OPS:
        if op.name == _FUSED_NAME:
            return op

    def _ref(in0, in1, c0, c1, c2):
        b = ((in0.astype(np.float32) - in1.astype(np.float32)) ** 2).astype(
            np.float32
        )
        return b, c0 + b.reshape(b.shape[0], -1).sum(axis=-1, keepdims=True)

    spec = Spec(body=sq(Src0 - Src1), accum=add, accum_init=C0, reference=_ref)
    row = dve_ops._CUSTOM_DVE_ROW_BASE + len(dve_ops.OPS)
    assert row < 0x20, "custom-DVE row field overflow"
    shas = {}
    for ver in ("v3", "v4"):
        uops = lower(spec, ver=ver)
        shas[ver] = DveOpSpec(
            name=_FUSED_NAME, opcode=row, uops=uops, rd1_en=True
        ).sha(ver)
    op = dve_ops.DveOp(_FUSED_NAME, spec, subdim=False, uops_sha=shas)
    dve_ops.OPS.append(op)
    dve_ops._SUB_OPCODE_FOR_NAME[_FUSED_NAME] = row
    dve_ops.CUSTOM_DVE_SPECS[_FUSED_NAME] = spec
    return op


def _parse_modes(split_mode):
    """'a16f17' -> interleaved list of 33 modes; or an explicit 33-char
    string of a/f."""
    import re

    if re.fullmatch(r"[af]{33}", split_mode):
        return list(split_mode)
    m = re.fullmatch(r"a(\d+)f(\d+)", split_mode)
    assert m, f"bad split_mode {split_mode}"
    counts = {"a": int(m.group(1)), "f": int(m.group(2))}
    assert sum(counts.values()) == NT
    modes, done = [], {k: 0 for k in counts}
    for w in range(NT):
        scores = {
            k: counts[k] * (w + 1) / NT - done[k]
            for k in counts
            if counts[k] > 0
        }
        k = max(scores, key=lambda kk: scores[kk])
        modes.append(k)
        done[k] += 1
    return modes


def _layout(modes):
    """Per-window column offsets in the staged S2 array, plus DMA groups
    of consecutive windows packed to <= GROUP_COLS columns."""
    offs, widths = [], []
    off = 0
    for m in modes:
        w = D if m == "a" else 2 * D
        offs.append(off)
        widths.append(w)
        off += w
    total = off
    groups = []  # (first_window, n_windows, col_off, col_width)
    i = 0
    while i < NT:
        # first group = one window so ACT's pipeline starts earliest
        cap = widths[0] if i == 0 else GROUP_COLS
        j = i
        width = 0
        while j < NT and width + widths[j] <= cap:
            width += widths[j]
            j += 1
        if j == i:  # single window wider than cap
            j = i + 1
            width = widths[i]
        groups.append((i, j - i, offs[i], width))
        i = j
    return offs, widths, total, groups


def _build_nc(split_mode=DEFAULT_SPLIT):
    import concourse.bass as bass  # noqa: F401
    import concourse.tile as tile
    from concourse import bacc, mybir

    fused = _register_fused_op()
    modes = _parse_modes(split_mode)
    offs, widths, total_cols, groups = _layout(modes)

    f32 = mybir.dt.float32
    f16 = mybir.dt.float16
    f8 = mybir.dt.float8e4
    nc = bacc.Bacc(
        "TRN2", target_bir_lowering=False, debug=False, num_devices=N_CORES
    )

    s2 = nc.declare_dram_parameter("s2", [128, total_cols], f8, isOutput=False)
    wshift = nc.declare_dram_parameter("wshift", [128, 128], f16, isOutput=False)
    wfix = nc.declare_dram_parameter("wfix", [128, 128], f16, isOutput=False)
    wf8 = nc.declare_dram_parameter("wf8", [128, 128], f8, isOutput=False)
    mw = nc.declare_dram_parameter("mw", [128, NT], f32, isOutput=False)
    maskt = nc.declare_dram_parameter("maskt", [128, 32], f32, isOutput=False)
    ones = nc.declare_dram_parameter("ones", [128, 1], f32, isOutput=False)
    out = nc.declare_dram_parameter("out", [1, 2], f32, isOutput=True)

    with tile.TileContext(nc) as tc:
        with (
            tc.tile_pool(name="consts", bufs=1) as consts,
            tc.tile_pool(name="xbpool", bufs=1) as xbpool,
            tc.tile_pool(name="scra", bufs=2) as scrapool,
            tc.tile_pool(name="psum", bufs=3, space="PSUM") as pspool,
            tc.tile_pool(name="psmall", bufs=1, space="PSUM") as psmall,
        ):
            # ones first: the ACT-table warm below gates on it, and the
            # warm must complete before the first Square streams
            ones_sb = consts.tile([128, 1], f32)
            nc.sync.dma_start(out=ones_sb, in_=ones[:, :])
            # dummy sqrt: loads the sqrt_and_others table (contains square
            # too) during startup instead of stalling the first Square
            warm = consts.tile([1, 1], f32)
            nc.scalar.sqrt(warm, ones_sb[0:1, 0:1])

            # prime the pipeline: first data group + matmul weights before
            # the tail-only consts
            w_f8 = consts.tile([128, 128], f8)
            nc.sync.dma_start(out=w_f8, in_=wf8[:, :])
            gtiles = []
            (w0_0, nwin_0, coff_0, cwidth_0) = groups[0]
            xb0 = xbpool.tile([128, cwidth_0], f8, name="xbg0")
            nc.sync.dma_start(out=xb0[:, :], in_=s2[:, coff_0 : coff_0 + cwidth_0])
            gtiles.append(xb0)
            (w0_1, nwin_1, coff_1, cwidth_1) = groups[1]
            xb1 = xbpool.tile([128, cwidth_1], f8, name="xbg1")
            nc.sync.dma_start(out=xb1[:, :], in_=s2[:, coff_1 : coff_1 + cwidth_1])
            gtiles.append(xb1)
            w_sb = consts.tile([128, 128], f16)
            wfix_sb = consts.tile([128, 128], f16)
            mw_sb = consts.tile([128, NT], f32)
            mask_sb = consts.tile([128, 32], f32)

            r_a = consts.tile([128, NT], f32)
            # row 127 of 'f' columns is never written by accum_out
            nc.gpsimd.memset(r_a, 0.0)
            e_sb = consts.tile([128, NT + 1], f16)
            nc.vector.memset(e_sb[:, NT : NT + 1], 0.0)
            g = consts.tile([128, 2], f32)

            scr_d = consts.tile([128, D], f16)   # fused-op elementwise out

            ps_d = psmall.tile([128, 512], f32, name="ps_d")
            for gidx, (w0, nwin, coff, cwidth) in enumerate(groups):
                if gidx < len(gtiles):
                    xb = gtiles[gidx]
                else:
                    xb = xbpool.tile([128, cwidth], f8, name=f"xbg{gidx}")
                    nc.sync.dma_start(
                        out=xb[:, :], in_=s2[:, coff : coff + cwidth]
                    )
                for k in range(nwin):
                    i = w0 + k
                    o = offs[i] - coff
                    xw = xb[:, o : o + D]
                    if modes[i] == "a":
                        ps = pspool.tile([128, D], f32)
                        nc.tensor.matmul(
                            ps[:, 0:512], lhsT=w_f8, rhs=xw[:, 0:512],
                            start=True, stop=True,
                        )
                        nc.tensor.matmul(
                            ps[:, 512:D], lhsT=w_f8, rhs=xw[:, 512:D],
                            start=True, stop=True,
                        )
                        scr = scrapool.tile([128, D], f16)
                        nc.scalar.activation(
                            scr,
                            ps,
                            mybir.ActivationFunctionType.Square,
                            accum_out=r_a[:, i : i + 1],
                        )
                    else:
                        xs = xb[:, o + D : o + 2 * D]
                        nc.vector._custom_dve(
                            fused,
                            out=scr_d[0:127, :],
                            in0=xs[0:127, :],
                            in1=xw[0:127, :],
                            s0=0.0,
                            s1=0.0,
                            imm2=0.0,
                            accum_out=r_a[0:127, i : i + 1],
                        )

            # tail-only consts: issued after all data groups so their DMA
            # configs don't delay the stream (needed only at ~35us)
            nc.sync.dma_start(out=w_sb, in_=wshift[:, :])
            nc.sync.dma_start(out=wfix_sb, in_=wfix[:, :])
            nc.sync.dma_start(out=mw_sb, in_=mw[:, :])
            nc.sync.dma_start(out=mask_sb, in_=maskt[:, :])
            nc.vector.tensor_reduce(
                g[:, 1:2], mask_sb, axis=mybir.AxisListType.X, op=mybir.AluOpType.add
            )

            # ---- tail: E = sqrt(R) (padded with a zero column) ----
            # fan-in: a single cheap DVE copy after all accum writers so
            # the sqrt waits on one cross-engine edge instead of 16
            r_j = consts.tile([128, NT], f32)
            nc.vector.tensor_copy(r_j, r_a)
            nc.scalar.activation(
                e_sb[:, 0:NT], r_j, mybir.ActivationFunctionType.Sqrt
            )

            # psD[j, i] = E[j+1, i] - E[j, i]; row 126 needs E[0, i+1]
            # (delta at the tile boundary) -- added by a second accumulating
            # matmul with wfix[0, 126] = 1 against the column-shifted E.
            # E[127, :] is all zeros so the first matmul contributes only
            # -E[126, i] to row 126.
            nc.tensor.matmul(
                ps_d[:, 0:NT], lhsT=w_sb, rhs=e_sb[:, 0:NT],
                start=True, stop=False,
            )
            nc.tensor.matmul(
                ps_d[:, 0:NT],
                lhsT=wfix_sb,
                rhs=e_sb[:, 1 : NT + 1],
                start=False,
                stop=True,
            )

            # fused relu(psD) * mw + free-dim reduce in one DVE op
            wt = consts.tile([128, NT], f32)
            nc.vector.scalar_tensor_tensor(
                out=wt,
                in0=ps_d[:, 0:NT],
                scalar=0.0,
                in1=mw_sb,
                op0=mybir.AluOpType.max,
                op1=mybir.AluOpType.mult,
                accum_out=g[:, 0:1],
            )

            out_sb = consts.tile([1, 2], f32)
            nc.gpsimd.tensor_reduce(
                out_sb, g, axis=mybir.AxisListType.C, op=mybir.AluOpType.add
            )
            nc.sync.dma_start(out=out[:, :], in_=out_sb)

    nc.compile()
    return nc


def _host_consts():
    w = np.zeros((128, 128), dtype=np.float32)
    for j in range(127):
        w[j, j] = -1.0
        w[j + 1, j] = 1.0
    wfix = np.zeros((128, 128), dtype=np.float32)
    wfix[0, 126] = 1.0
    ones = np.ones((128, 1), dtype=np.float32)
    return w, wfix, ones


def _per_core_inputs(states_b, mask_b, rp_b, w, wfix, ones, modes, offs,
                     total_cols):
    # weighted-sum coefficients: mw[p, i] = mask[t+2] * weight[t], t = 127i+p
    t = np.arange(L - 2, dtype=np.float64)
    dist = np.maximum(float(rp_b) - t - 2.0, 0.0)
    weight = np.where(dist < 5, 2.0 + (5.0 - dist) * 0.5, 1.0).astype(np.float32)
    mwvec = (mask_b[2:L] * weight).astype(np.float32)  # [L-2]
    vals = np.zeros(NT * STRIDE, dtype=np.float32)
    vals[: L - 2] = mwvec
    mw = np.zeros((128, NT), dtype=np.float32)
    mw[:STRIDE, :] = vals.reshape(NT, STRIDE).T

    mt = mask_b.astype(np.float32).copy()
    mt[0:2] = 0.0
    maskt = mt.reshape(128, 32)

    # window-major fp8 staging; 'f' windows also stage the one-token-
    # shifted copy so the DVE subtract is partition-aligned
    sf8 = states_b.astype(ml_dtypes.float8_e4m3)
    s2 = np.zeros((128, total_cols), dtype=ml_dtypes.float8_e4m3)
    for i, m in enumerate(modes):
        r0 = STRIDE * i
        rows = min(128, L - r0)
        o = offs[i]
        s2[:rows, o : o + D] = sf8[r0 : r0 + rows]
        if m == "f":
            rows2 = min(128, L - r0 - 1)
            s2[:rows2, o + D : o + 2 * D] = sf8[r0 + 1 : r0 + 1 + rows2]

    return {
        "s2": s2,
        "wshift": w.astype(np.float16),
        "wfix": wfix.astype(np.float16),
        "wf8": w.astype(ml_dtypes.float8_e4m3),
        "mw": mw,
        "maskt": np.ascontiguousarray(maskt),
        "ones": ones,
    }


def _get_nc(split_mode=DEFAULT_SPLIT):
    key = ("nc", split_mode)
    if key not in _CACHE:
        _CACHE[key] = _build_nc(split_mode)
    return _CACHE[key]


def _run(states, reasoning_mask, result_token_positions, trace=False,
         split_mode=DEFAULT_SPLIT):
    from concourse.bass_utils import run_bass_kernel_spmd

    states = np.asarray(states, dtype=np.float32)
    mask = np.asarray(reasoning_mask, dtype=np.float32)
    rp = np.asarray(result_token_positions)

    modes = _parse_modes(split_mode)
    offs, widths, total_cols, groups = _layout(modes)
    w, wfix, ones = _host_consts()
    in_maps = [
        _per_core_inputs(
            states[b], mask[b], rp[b], w, wfix, ones, modes, offs, total_cols
        )
        for b in range(N_CORES)
    ]
    nc = _get_nc(split_mode)
    res = run_bass_kernel_spmd(
        nc, in_maps, core_ids=list(range(N_CORES)), trace=trace
    )
    partials = np.stack([res.results[i]["out"][0] for i in range(N_CORES)])  # [8, 2]
    s = partials[:, 0].astype(np.float64).sum()
    m = partials[:, 1].astype(np.float64).sum()
    value = np.float32(s / (m + 1e-9))
    return value, res


def kernel(states, reasoning_mask, result_token_positions):
    value, _ = _run(states, reasoning_mask, result_token_positions)
    return np.asarray(value, dtype=np.float32)


# revision 41
# speedup vs baseline: 1.0427x; 1.0427x over previous
"""ApproachLoss kernel for 8 TRN2 NeuronCores (Bass/Tile), fp8 edition.

Reference computation (per batch element b):
    deltas[t]  = ||states[b, t+1] - states[b, t]||          t in [0, L-2]
    di[j]      = relu(deltas[j+1] - deltas[j])              j in [0, L-3]
    weighted   = di * reasoning_mask[b, 2:] * approach_weight
    loss       = sum_b sum_j weighted / (sum_b sum_t mask[b, 2:] + 1e-9)

Sharding: pure data-parallel, batch element b -> core b. Each core returns
[weighted_sum_b, mask_sum_b]; the host sums the 16 scalars and divides.

Host staging: states are quantized to fp8 e4m3 (loss rel-err 6.5e-4, well
under the 2e-2 gate; quartering HBM traffic vs f32) and laid out
window-major for 33 token windows of 128 tokens, stride 127 (1-token
overlap so every adjacent-token pair lands inside some window). Windows
are packed into ~3KB-per-partition DMA groups (finer groups shorten
per-window data-availability latency; group completion gates its
windows' compute), one SBUF buffer per group
(no pool recycling - recycling coupled DMA issue to compute completion
and stalled the stream). All data DMAs go on the SP HWDGE queue; SWDGE
(gpsimd) starts later on this system and throttles the stream.

Per-window engine assignment (a17f16 balances ACT vs DVE busy time;
measured per-window costs: ACT square+accum ~1.19us, DVE fused op
~1.23us, PE matmul pair ~1.10us - DVE 2x/4x modes do not engage on
this toolchain, and GpSimd supports no useful op here):
  mode 'a': S2 block [x] (1024 cols). TensorE shift-difference matmul
            (stationary fp8 W, W[j,j]=-1, W[j+1,j]=+1, exact in e4m3)
            -> PSUM diff [128, 1024] f32 -> ScalarE activation(Square,
            accum_out) into R[:, i].
  mode 'f': S2 block [x | x_shifted] (2048 cols; the shifted copy is
            staged host-side: compute engines cannot read
            partition-offset slices - birverifier rejects them - and
            SBUF->SBUF DMA shift copies serialize on one DMA engine).
            One fused custom-DVE op (SUB_SQ_REDUCE_ANT, registered at
            build time via the sanctioned concourse.dve_ops framework;
            ucode table ships inside the NEFF) computes (xs - xw)^2
            with free-dim accumulate into R[:, i] in a single DVE
            instruction - this replaced separate subtract + square
            passes (2.35us -> 1.23us per window).
Tail: E = sqrt(R) -> delta_increase via shift matmul + boundary-fix
matmul (wfix[0,126] = 1 against the column-shifted E; E[127, :] = 0 so
the first matmul contributes only -E[126, i] to row 126) -> fused
relu * (mask*weight) + reduce -> GpSimd tensor_reduce(axis=C)
partition reduction -> out [1, 2] = [weighted_sum, mask_sum].

Measured: ~39.3-41 us HW exec on a quiet device, up to ~49 when the
tunneled device is noisy (best 39.26; baseline f32 kernel: 65 us).
Tail-only const DMAs (wshift/wfix/mw/maskt) are issued AFTER all data
groups: their 4 DGE configs otherwise sit between groups 1 and 2 on the
SP queue and delay every later group's config by ~2.3us (-1.3us).
Startup ~11 us (framework preamble ~7 us, HWDGE queue starts at
~8.2 us; the ACT sqrt/square table warms during it), streaming phase
~21-24 us with ACT and DVE saturated and PE ~70% (PE stays at the
1.2 GHz pstate - pre-ramping with dummy matmuls does not stick),
tail ~5 us (drain + sqrt chain + Tile end barrier). Failed routes:
mid-stream tail split, SWDGE data DMAs, single-window DMA priming,
PE warm-up, DVE 2x modes - see project memory.
"""

from operator import add

import ml_dtypes
import numpy as np

B, L, D = 8, 4096, 1024
NT = 33          # diff tiles: tile i covers tokens 127i .. 127i+127
STRIDE = 127     # valid diffs per full tile
N_CORES = 8
GROUP_COLS = 3 * 1024   # max cols per DMA group tile

_CACHE = {}

DEFAULT_SPLIT = "a17f16"

_FUSED_NAME = "SUB_SQ_REDUCE_ANT"


def _register_fused_op():
    """Register the fused custom-DVE op  out = (in0-in1)^2,
    accum_out = c0 + sum(out)  via the sanctioned custom-DVE framework
    (concourse.dve_ops). The op's ucode table is generated at compile
    time and ships inside the NEFF."""
    from concourse import dve_ops
    from concourse.dve_spec import C0, Spec, Src0, Src1, lower, sq
    from concourse.dve_uop import DveOpSpec

    for op in dve_ops.OPS:
        if op.name == _FUSED_NAME:
            return op

    def _ref(in0, in1, c0, c1, c2):
        b = ((in0.astype(np.float32) - in1.astype(np.float32)) ** 2).astype(
            np.float32
        )
        return b, c0 + b.reshape(b.shape[0], -1).sum(axis=-1, keepdims=True)

    spec = Spec(body=sq(Src0 - Src1), accum=add, accum_init=C0, reference=_ref)
    row = dve_ops._CUSTOM_DVE_ROW_BASE + len(dve_ops.OPS)
    assert row < 0x20, "custom-DVE row field overflow"
    shas = {}
    for ver in ("v3", "v4"):
        uops = lower(spec, ver=ver)
        shas[ver] = DveOpSpec(
            name=_FUSED_NAME, opcode=row, uops=uops, rd1_en=True
        ).sha(ver)
    op = dve_ops.DveOp(_FUSED_NAME, spec, subdim=False, uops_sha=shas)
    dve_ops.OPS.append(op)
    dve_ops._SUB_OPCODE_FOR_NAME[_FUSED_NAME] = row
    dve_ops.CUSTOM_DVE_SPECS[_FUSED_NAME] = spec
    return op


def _parse_modes(split_mode):
    """'a16f17' -> interleaved list of 33 modes; or an explicit 33-char
    string of a/f."""
    import re

    if re.fullmatch(r"[af]{33}", split_mode):
        return list(split_mode)
    m = re.fullmatch(r"a(\d+)f(\d+)", split_mode)
    assert m, f"bad split_mode {split_mode}"
    counts = {"a": int(m.group(1)), "f": int(m.group(2))}
    assert sum(counts.values()) == NT
    modes, done = [], {k: 0 for k in counts}
    for w in range(NT):
        scores = {
            k: counts[k] * (w + 1) / NT - done[k]
            for k in counts
            if counts[k] > 0
        }
        k = max(scores, key=lambda kk: scores[kk])
        modes.append(k)
        done[k] += 1
    return modes


def _layout(modes):
    """Per-window column offsets in the staged S2 array, plus DMA groups
    of consecutive windows packed to <= GROUP_COLS columns."""
    offs, widths = [], []
    off = 0
    for m in modes:
        w = D if m == "a" else 2 * D
        offs.append(off)
        widths.append(w)
        off += w
    total = off
    groups = []  # (first_window, n_windows, col_off, col_width)
    i = 0
    while i < NT:
        # first group = one window so ACT's pipeline starts earliest
        cap = widths[0] if i == 0 else GROUP_COLS
        j = i
        width = 0
        while j < NT and width + widths[j] <= cap:
            width += widths[j]
            j += 1
        if j == i:  # single window wider than cap
            j = i + 1
            width = widths[i]
        groups.append((i, j - i, offs[i], width))
        i = j
    return offs, widths, total, groups


def _build_nc(split_mode=DEFAULT_SPLIT):
    import concourse.bass as bass  # noqa: F401
    import concourse.tile as tile
    from concourse import bacc, mybir

    fused = _register_fused_op()
    modes = _parse_modes(split_mode)
    offs, widths, total_cols, groups = _layout(modes)

    f32 = mybir.dt.float32
    f16 = mybir.dt.float16
    f8 = mybir.dt.float8e4
    nc = bacc.Bacc(
        "TRN2", target_bir_lowering=False, debug=False, num_devices=N_CORES
    )

    s2 = nc.declare_dram_parameter("s2", [128, total_cols], f8, isOutput=False)
    wshift = nc.declare_dram_parameter("wshift", [128, 128], f16, isOutput=False)
    wfix = nc.declare_dram_parameter("wfix", [128, 128], f16, isOutput=False)
    wf8 = nc.declare_dram_parameter("wf8", [128, 128], f8, isOutput=False)
    mw = nc.declare_dram_parameter("mw", [128, NT], f32, isOutput=False)
    maskt = nc.declare_dram_parameter("maskt", [128, 32], f32, isOutput=False)
    out = nc.declare_dram_parameter("out", [1, 2], f32, isOutput=True)

    with tile.TileContext(nc) as tc:
        with (
            tc.tile_pool(name="consts", bufs=1) as consts,
            tc.tile_pool(name="xbpool", bufs=1) as xbpool,
            tc.tile_pool(name="scra", bufs=2) as scrapool,
            tc.tile_pool(name="psum", bufs=3, space="PSUM") as pspool,
            tc.tile_pool(name="psmall", bufs=1, space="PSUM") as psmall,
        ):
            # dummy sqrt: loads the sqrt_and_others table (contains square
            # too) during startup instead of stalling the first Square.
            # Input is a memset tile, not a DMA'd const: no DMA dependency,
            # and one fewer config ahead of the data groups on the SP queue
            wsrc = consts.tile([1, 1], f32)
            nc.gpsimd.memset(wsrc, 1.0)
            warm = consts.tile([1, 1], f32)
            nc.scalar.sqrt(warm, wsrc)

            # prime the pipeline: first data group + matmul weights before
            # the tail-only consts
            w_f8 = consts.tile([128, 128], f8)
            nc.sync.dma_start(out=w_f8, in_=wf8[:, :])
            gtiles = []
            (w0_0, nwin_0, coff_0, cwidth_0) = groups[0]
            xb0 = xbpool.tile([128, cwidth_0], f8, name="xbg0")
            nc.sync.dma_start(out=xb0[:, :], in_=s2[:, coff_0 : coff_0 + cwidth_0])
            gtiles.append(xb0)
            (w0_1, nwin_1, coff_1, cwidth_1) = groups[1]
            xb1 = xbpool.tile([128, cwidth_1], f8, name="xbg1")
            nc.sync.dma_start(out=xb1[:, :], in_=s2[:, coff_1 : coff_1 + cwidth_1])
            gtiles.append(xb1)
            w_sb = consts.tile([128, 128], f16)
            wfix_sb = consts.tile([128, 128], f16)
            mw_sb = consts.tile([128, NT], f32)
            mask_sb = consts.tile([128, 32], f32)

            r_a = consts.tile([128, NT], f32)
            # row 127 of 'f' columns is never written by accum_out
            nc.gpsimd.memset(r_a, 0.0)
            e_sb = consts.tile([128, NT + 1], f16)
            nc.vector.memset(e_sb[:, NT : NT + 1], 0.0)
            g = consts.tile([128, 2], f32)

            scr_d = consts.tile([128, D], f16)   # fused-op elementwise out

            ps_d = psmall.tile([128, 512], f32, name="ps_d")
            for gidx, (w0, nwin, coff, cwidth) in enumerate(groups):
                if gidx < len(gtiles):
                    xb = gtiles[gidx]
                else:
                    xb = xbpool.tile([128, cwidth], f8, name=f"xbg{gidx}")
                    nc.sync.dma_start(
                        out=xb[:, :], in_=s2[:, coff : coff + cwidth]
                    )
                for k in range(nwin):
                    i = w0 + k
                    o = offs[i] - coff
                    xw = xb[:, o : o + D]
                    if modes[i] == "a":
                        ps = pspool.tile([128, D], f32)
                        nc.tensor.matmul(
                            ps[:, 0:512], lhsT=w_f8, rhs=xw[:, 0:512],
                            start=True, stop=True,
                        )
                        nc.tensor.matmul(
                            ps[:, 512:D], lhsT=w_f8, rhs=xw[:, 512:D],
                            start=True, stop=True,
                        )
                        scr = scrapool.tile([128, D], f16)
                        nc.scalar.activation(
                            scr,
                            ps,
                            mybir.ActivationFunctionType.Square,
                            accum_out=r_a[:, i : i + 1],
                        )
                    else:
                        xs = xb[:, o + D : o + 2 * D]
                        nc.vector._custom_dve(
                            fused,
                            out=scr_d[0:127, :],
                            in0=xs[0:127, :],
                            in1=xw[0:127, :],
                            s0=0.0,
                            s1=0.0,
                            imm2=0.0,
                            accum_out=r_a[0:127, i : i + 1],
                        )

            # tail-only consts: issued after all data groups so their DMA
            # configs don't delay the stream (needed only at ~35us)
            nc.sync.dma_start(out=w_sb, in_=wshift[:, :])
            nc.sync.dma_start(out=wfix_sb, in_=wfix[:, :])
            nc.sync.dma_start(out=mw_sb, in_=mw[:, :])
            nc.sync.dma_start(out=mask_sb, in_=maskt[:, :])
            nc.vector.tensor_reduce(
                g[:, 1:2], mask_sb, axis=mybir.AxisListType.X, op=mybir.AluOpType.add
            )

            # ---- tail: E = sqrt(R) (padded with a zero column) ----
            # fan-in: a single cheap DVE copy after all accum writers so
            # the sqrt waits on one cross-engine edge instead of 16
            r_j = consts.tile([128, NT], f32)
            nc.vector.tensor_copy(r_j, r_a)
            nc.scalar.activation(
                e_sb[:, 0:NT], r_j, mybir.ActivationFunctionType.Sqrt
            )

            # psD[j, i] = E[j+1, i] - E[j, i]; row 126 needs E[0, i+1]
            # (delta at the tile boundary) -- added by a second accumulating
            # matmul with wfix[0, 126] = 1 against the column-shifted E.
            # E[127, :] is all zeros so the first matmul contributes only
            # -E[126, i] to row 126.
            nc.tensor.matmul(
                ps_d[:, 0:NT], lhsT=w_sb, rhs=e_sb[:, 0:NT],
                start=True, stop=False,
            )
            nc.tensor.matmul(
                ps_d[:, 0:NT],
                lhsT=wfix_sb,
                rhs=e_sb[:, 1 : NT + 1],
                start=False,
                stop=True,
            )

            # fused relu(psD) * mw + free-dim reduce in one DVE op
            wt = consts.tile([128, NT], f32)
            nc.vector.scalar_tensor_tensor(
                out=wt,
                in0=ps_d[:, 0:NT],
                scalar=0.0,
                in1=mw_sb,
                op0=mybir.AluOpType.max,
                op1=mybir.AluOpType.mult,
                accum_out=g[:, 0:1],
            )

            out_sb = consts.tile([1, 2], f32)
            nc.gpsimd.tensor_reduce(
                out_sb, g, axis=mybir.AxisListType.C, op=mybir.AluOpType.add
            )
            nc.sync.dma_start(out=out[:, :], in_=out_sb)

    nc.compile()
    return nc


def _host_consts():
    w = np.zeros((128, 128), dtype=np.float32)
    for j in range(127):
        w[j, j] = -1.0
        w[j + 1, j] = 1.0
    wfix = np.zeros((128, 128), dtype=np.float32)
    wfix[0, 126] = 1.0
    return w, wfix


def _per_core_inputs(states_b, mask_b, rp_b, w, wfix, modes, offs,
                     total_cols):
    # weighted-sum coefficients: mw[p, i] = mask[t+2] * weight[t], t = 127i+p
    t = np.arange(L - 2, dtype=np.float64)
    dist = np.maximum(float(rp_b) - t - 2.0, 0.0)
    weight = np.where(dist < 5, 2.0 + (5.0 - dist) * 0.5, 1.0).astype(np.float32)
    mwvec = (mask_b[2:L] * weight).astype(np.float32)  # [L-2]
    vals = np.zeros(NT * STRIDE, dtype=np.float32)
    vals[: L - 2] = mwvec
    mw = np.zeros((128, NT), dtype=np.float32)
    mw[:STRIDE, :] = vals.reshape(NT, STRIDE).T

    mt = mask_b.astype(np.float32).copy()
    mt[0:2] = 0.0
    maskt = mt.reshape(128, 32)

    # window-major fp8 staging; 'f' windows also stage the one-token-
    # shifted copy so the DVE subtract is partition-aligned
    sf8 = states_b.astype(ml_dtypes.float8_e4m3)
    s2 = np.zeros((128, total_cols), dtype=ml_dtypes.float8_e4m3)
    for i, m in enumerate(modes):
        r0 = STRIDE * i
        rows = min(128, L - r0)
        o = offs[i]
        s2[:rows, o : o + D] = sf8[r0 : r0 + rows]
        if m == "f":
            rows2 = min(128, L - r0 - 1)
            s2[:rows2, o + D : o + 2 * D] = sf8[r0 + 1 : r0 + 1 + rows2]

    return {
        "s2": s2,
        "wshift": w.astype(np.float16),
        "wfix": wfix.astype(np.float16),
        "wf8": w.astype(ml_dtypes.float8_e4m3),
        "mw": mw,
        "maskt": np.ascontiguousarray(maskt),
    }


def _get_nc(split_mode=DEFAULT_SPLIT):
    key = ("nc", split_mode)
    if key not in _CACHE:
        _CACHE[key] = _build_nc(split_mode)
    return _CACHE[key]


def _run(states, reasoning_mask, result_token_positions, trace=False,
         split_mode=DEFAULT_SPLIT):
    from concourse.bass_utils import run_bass_kernel_spmd

    states = np.asarray(states, dtype=np.float32)
    mask = np.asarray(reasoning_mask, dtype=np.float32)
    rp = np.asarray(result_token_positions)

    modes = _parse_modes(split_mode)
    offs, widths, total_cols, groups = _layout(modes)
    w, wfix = _host_consts()
    in_maps = [
        _per_core_inputs(
            states[b], mask[b], rp[b], w, wfix, modes, offs, total_cols
        )
        for b in range(N_CORES)
    ]
    nc = _get_nc(split_mode)
    res = run_bass_kernel_spmd(
        nc, in_maps, core_ids=list(range(N_CORES)), trace=trace
    )
    partials = np.stack([res.results[i]["out"][0] for i in range(N_CORES)])  # [8, 2]
    s = partials[:, 0].astype(np.float64).sum()
    m = partials[:, 1].astype(np.float64).sum()
    value = np.float32(s / (m + 1e-9))
    return value, res


def kernel(states, reasoning_mask, result_token_positions):
    value, _ = _run(states, reasoning_mask, result_token_positions)
    return np.asarray(value, dtype=np.float32)


# revision 42
# speedup vs baseline: 1.0430x; 1.0003x over previous
"""ApproachLoss kernel for 8 TRN2 NeuronCores (Bass/Tile), fp8 edition.

Reference computation (per batch element b):
    deltas[t]  = ||states[b, t+1] - states[b, t]||          t in [0, L-2]
    di[j]      = relu(deltas[j+1] - deltas[j])              j in [0, L-3]
    weighted   = di * reasoning_mask[b, 2:] * approach_weight
    loss       = sum_b sum_j weighted / (sum_b sum_t mask[b, 2:] + 1e-9)

Sharding: pure data-parallel, batch element b -> core b. Each core returns
[weighted_sum_b, mask_sum_b]; the host sums the 16 scalars and divides.

Host staging: states are quantized to fp8 e4m3 (loss rel-err 6.5e-4, well
under the 2e-2 gate; quartering HBM traffic vs f32) and laid out
window-major for 33 token windows of 128 tokens, stride 127 (1-token
overlap so every adjacent-token pair lands inside some window). Windows
are packed into ~3KB-per-partition DMA groups (finer groups shorten
per-window data-availability latency; group completion gates its
windows' compute), one SBUF buffer per group
(no pool recycling - recycling coupled DMA issue to compute completion
and stalled the stream). All data DMAs go on the SP HWDGE queue; SWDGE
(gpsimd) starts later on this system and throttles the stream.

Per-window engine assignment (a17f16 balances ACT vs DVE busy time;
measured per-window costs: ACT square+accum ~1.19us, DVE fused op
~1.23us, PE matmul pair ~1.10us - DVE 2x/4x modes do not engage on
this toolchain, and GpSimd supports no useful op here):
  mode 'a': S2 block [x] (1024 cols). TensorE shift-difference matmul
            (stationary fp8 W, W[j,j]=-1, W[j+1,j]=+1, exact in e4m3)
            -> PSUM diff [128, 1024] f32 -> ScalarE activation(Square,
            accum_out) into R[:, i].
  mode 'f': S2 block [x | x_shifted] (2048 cols; the shifted copy is
            staged host-side: compute engines cannot read
            partition-offset slices - birverifier rejects them - and
            SBUF->SBUF DMA shift copies serialize on one DMA engine).
            One fused custom-DVE op (SUB_SQ_REDUCE_ANT, registered at
            build time via the sanctioned concourse.dve_ops framework;
            ucode table ships inside the NEFF) computes (xs - xw)^2
            with free-dim accumulate into R[:, i] in a single DVE
            instruction - this replaced separate subtract + square
            passes (2.35us -> 1.23us per window).
Tail: E = sqrt(R) -> delta_increase via shift matmul + boundary-fix
matmul (wfix[0,126] = 1 against the column-shifted E; E[127, :] = 0 so
the first matmul contributes only -E[126, i] to row 126) -> fused
relu * (mask*weight) + reduce -> GpSimd tensor_reduce(axis=C)
partition reduction -> out [1, 2] = [weighted_sum, mask_sum].

Measured: ~39.3-41 us HW exec on a quiet device, up to ~49 when the
tunneled device is noisy (best 39.26; baseline f32 kernel: 65 us).
Tail-only const DMAs (wshift/wfix/mw/maskt) are issued AFTER all data
groups: their 4 DGE configs otherwise sit between groups 1 and 2 on the
SP queue and delay every later group's config by ~2.3us (-1.3us).
Startup ~11 us (framework preamble ~7 us, HWDGE queue starts at
~8.2 us; the ACT sqrt/square table warms during it), streaming phase
~21-24 us with ACT and DVE saturated and PE ~70% (PE stays at the
1.2 GHz pstate - pre-ramping with dummy matmuls does not stick),
tail ~5 us (drain + sqrt chain + Tile end barrier). Failed routes:
mid-stream tail split, SWDGE data DMAs, single-window DMA priming,
PE warm-up, DVE 2x modes - see project memory.
"""

from operator import add

import ml_dtypes
import numpy as np

B, L, D = 8, 4096, 1024
NT = 33          # diff tiles: tile i covers tokens 127i .. 127i+127
STRIDE = 127     # valid diffs per full tile
N_CORES = 8
GROUP_COLS = 3 * 1024   # max cols per DMA group tile

_CACHE = {}

DEFAULT_SPLIT = "fa" * 16 + "a"  # f-first: group 0 needs no weights

_FUSED_NAME = "SUB_SQ_REDUCE_ANT"


def _register_fused_op():
    """Register the fused custom-DVE op  out = (in0-in1)^2,
    accum_out = c0 + sum(out)  via the sanctioned custom-DVE framework
    (concourse.dve_ops). The op's ucode table is generated at compile
    time and ships inside the NEFF."""
    from concourse import dve_ops
    from concourse.dve_spec import C0, Spec, Src0, Src1, lower, sq
    from concourse.dve_uop import DveOpSpec

    for op in dve_ops.OPS:
        if op.name == _FUSED_NAME:
            return op

    def _ref(in0, in1, c0, c1, c2):
        b = ((in0.astype(np.float32) - in1.astype(np.float32)) ** 2).astype(
            np.float32
        )
        return b, c0 + b.reshape(b.shape[0], -1).sum(axis=-1, keepdims=True)

    spec = Spec(body=sq(Src0 - Src1), accum=add, accum_init=C0, reference=_ref)
    row = dve_ops._CUSTOM_DVE_ROW_BASE + len(dve_ops.OPS)
    assert row < 0x20, "custom-DVE row field overflow"
    shas = {}
    for ver in ("v3", "v4"):
        uops = lower(spec, ver=ver)
        shas[ver] = DveOpSpec(
            name=_FUSED_NAME, opcode=row, uops=uops, rd1_en=True
        ).sha(ver)
    op = dve_ops.DveOp(_FUSED_NAME, spec, subdim=False, uops_sha=shas)
    dve_ops.OPS.append(op)
    dve_ops._SUB_OPCODE_FOR_NAME[_FUSED_NAME] = row
    dve_ops.CUSTOM_DVE_SPECS[_FUSED_NAME] = spec
    return op


def _parse_modes(split_mode):
    """'a16f17' -> interleaved list of 33 modes; or an explicit 33-char
    string of a/f."""
    import re

    if re.fullmatch(r"[af]{33}", split_mode):
        return list(split_mode)
    m = re.fullmatch(r"a(\d+)f(\d+)", split_mode)
    assert m, f"bad split_mode {split_mode}"
    counts = {"a": int(m.group(1)), "f": int(m.group(2))}
    assert sum(counts.values()) == NT
    modes, done = [], {k: 0 for k in counts}
    for w in range(NT):
        scores = {
            k: counts[k] * (w + 1) / NT - done[k]
            for k in counts
            if counts[k] > 0
        }
        k = max(scores, key=lambda kk: scores[kk])
        modes.append(k)
        done[k] += 1
    return modes


def _layout(modes):
    """Per-window column offsets in the staged S2 array, plus DMA groups
    of consecutive windows packed to <= GROUP_COLS columns."""
    offs, widths = [], []
    off = 0
    for m in modes:
        w = D if m == "a" else 2 * D
        offs.append(off)
        widths.append(w)
        off += w
    total = off
    groups = []  # (first_window, n_windows, col_off, col_width)
    i = 0
    while i < NT:
        # first group = one window so ACT's pipeline starts earliest
        cap = widths[0] if i == 0 else GROUP_COLS
        j = i
        width = 0
        while j < NT and width + widths[j] <= cap:
            width += widths[j]
            j += 1
        if j == i:  # single window wider than cap
            j = i + 1
            width = widths[i]
        groups.append((i, j - i, offs[i], width))
        i = j
    return offs, widths, total, groups


def _build_nc(split_mode=DEFAULT_SPLIT):
    import concourse.bass as bass  # noqa: F401
    import concourse.tile as tile
    from concourse import bacc, mybir

    fused = _register_fused_op()
    modes = _parse_modes(split_mode)
    offs, widths, total_cols, groups = _layout(modes)

    f32 = mybir.dt.float32
    f16 = mybir.dt.float16
    f8 = mybir.dt.float8e4
    nc = bacc.Bacc(
        "TRN2", target_bir_lowering=False, debug=False, num_devices=N_CORES
    )

    s2 = nc.declare_dram_parameter("s2", [128, total_cols], f8, isOutput=False)
    wshift = nc.declare_dram_parameter("wshift", [128, 128], f16, isOutput=False)
    wfix = nc.declare_dram_parameter("wfix", [128, 128], f16, isOutput=False)
    wf8 = nc.declare_dram_parameter("wf8", [128, 128], f8, isOutput=False)
    mw = nc.declare_dram_parameter("mw", [128, NT], f32, isOutput=False)
    maskt = nc.declare_dram_parameter("maskt", [128, 32], f32, isOutput=False)
    out = nc.declare_dram_parameter("out", [1, 2], f32, isOutput=True)

    with tile.TileContext(nc) as tc:
        with (
            tc.tile_pool(name="consts", bufs=1) as consts,
            tc.tile_pool(name="xbpool", bufs=1) as xbpool,
            tc.tile_pool(name="scra", bufs=2) as scrapool,
            tc.tile_pool(name="psum", bufs=3, space="PSUM") as pspool,
            tc.tile_pool(name="psmall", bufs=1, space="PSUM") as psmall,
        ):
            # dummy sqrt: loads the sqrt_and_others table (contains square
            # too) during startup instead of stalling the first Square.
            # Input is a memset tile, not a DMA'd const: no DMA dependency,
            # and one fewer config ahead of the data groups on the SP queue
            wsrc = consts.tile([1, 1], f32)
            nc.gpsimd.memset(wsrc, 1.0)
            warm = consts.tile([1, 1], f32)
            nc.scalar.sqrt(warm, wsrc)

            # prime the pipeline: first data group + matmul weights before
            # the tail-only consts
            gtiles = []
            (w0_0, nwin_0, coff_0, cwidth_0) = groups[0]
            xb0 = xbpool.tile([128, cwidth_0], f8, name="xbg0")
            nc.sync.dma_start(out=xb0[:, :], in_=s2[:, coff_0 : coff_0 + cwidth_0])
            gtiles.append(xb0)
            w_f8 = consts.tile([128, 128], f8)
            nc.sync.dma_start(out=w_f8, in_=wf8[:, :])
            (w0_1, nwin_1, coff_1, cwidth_1) = groups[1]
            xb1 = xbpool.tile([128, cwidth_1], f8, name="xbg1")
            nc.sync.dma_start(out=xb1[:, :], in_=s2[:, coff_1 : coff_1 + cwidth_1])
            gtiles.append(xb1)
            w_sb = consts.tile([128, 128], f16)
            wfix_sb = consts.tile([128, 128], f16)
            mw_sb = consts.tile([128, NT], f32)
            mask_sb = consts.tile([128, 32], f32)

            r_a = consts.tile([128, NT], f32)
            # row 127 of 'f' columns is never written by accum_out
            nc.gpsimd.memset(r_a, 0.0)
            e_sb = consts.tile([128, NT + 1], f16)
            nc.vector.memset(e_sb[:, NT : NT + 1], 0.0)
            g = consts.tile([128, 2], f32)

            scr_d = consts.tile([128, D], f16)   # fused-op elementwise out

            ps_d = psmall.tile([128, 512], f32, name="ps_d")
            for gidx, (w0, nwin, coff, cwidth) in enumerate(groups):
                if gidx < len(gtiles):
                    xb = gtiles[gidx]
                else:
                    xb = xbpool.tile([128, cwidth], f8, name=f"xbg{gidx}")
                    nc.sync.dma_start(
                        out=xb[:, :], in_=s2[:, coff : coff + cwidth]
                    )
                for k in range(nwin):
                    i = w0 + k
                    o = offs[i] - coff
                    xw = xb[:, o : o + D]
                    if modes[i] == "a":
                        ps = pspool.tile([128, D], f32)
                        nc.tensor.matmul(
                            ps[:, 0:512], lhsT=w_f8, rhs=xw[:, 0:512],
                            start=True, stop=True,
                        )
                        nc.tensor.matmul(
                            ps[:, 512:D], lhsT=w_f8, rhs=xw[:, 512:D],
                            start=True, stop=True,
                        )
                        scr = scrapool.tile([128, D], f16)
                        nc.scalar.activation(
                            scr,
                            ps,
                            mybir.ActivationFunctionType.Square,
                            accum_out=r_a[:, i : i + 1],
                        )
                    else:
                        xs = xb[:, o + D : o + 2 * D]
                        nc.vector._custom_dve(
                            fused,
                            out=scr_d[0:127, :],
                            in0=xs[0:127, :],
                            in1=xw[0:127, :],
                            s0=0.0,
                            s1=0.0,
                            imm2=0.0,
                            accum_out=r_a[0:127, i : i + 1],
                        )

            # tail-only consts: issued after all data groups so their DMA
            # configs don't delay the stream (needed only at ~35us)
            nc.sync.dma_start(out=w_sb, in_=wshift[:, :])
            nc.sync.dma_start(out=wfix_sb, in_=wfix[:, :])
            nc.sync.dma_start(out=mw_sb, in_=mw[:, :])
            nc.sync.dma_start(out=mask_sb, in_=maskt[:, :])
            nc.vector.tensor_reduce(
                g[:, 1:2], mask_sb, axis=mybir.AxisListType.X, op=mybir.AluOpType.add
            )

            # ---- tail: E = sqrt(R) (padded with a zero column) ----
            # fan-in: a single cheap DVE copy after all accum writers so
            # the sqrt waits on one cross-engine edge instead of 16
            r_j = consts.tile([128, NT], f32)
            nc.vector.tensor_copy(r_j, r_a)
            nc.scalar.activation(
                e_sb[:, 0:NT], r_j, mybir.ActivationFunctionType.Sqrt
            )

            # psD[j, i] = E[j+1, i] - E[j, i]; row 126 needs E[0, i+1]
            # (delta at the tile boundary) -- added by a second accumulating
            # matmul with wfix[0, 126] = 1 against the column-shifted E.
            # E[127, :] is all zeros so the first matmul contributes only
            # -E[126, i] to row 126.
            nc.tensor.matmul(
                ps_d[:, 0:NT], lhsT=w_sb, rhs=e_sb[:, 0:NT],
                start=True, stop=False,
            )
            nc.tensor.matmul(
                ps_d[:, 0:NT],
                lhsT=wfix_sb,
                rhs=e_sb[:, 1 : NT + 1],
                start=False,
                stop=True,
            )

            # fused relu(psD) * mw + free-dim reduce in one DVE op
            wt = consts.tile([128, NT], f32)
            nc.vector.scalar_tensor_tensor(
                out=wt,
                in0=ps_d[:, 0:NT],
                scalar=0.0,
                in1=mw_sb,
                op0=mybir.AluOpType.max,
                op1=mybir.AluOpType.mult,
                accum_out=g[:, 0:1],
            )

            out_sb = consts.tile([1, 2], f32)
            nc.gpsimd.tensor_reduce(
                out_sb, g, axis=mybir.AxisListType.C, op=mybir.AluOpType.add
            )
            nc.sync.dma_start(out=out[:, :], in_=out_sb)

    nc.compile()
    return nc


def _host_consts():
    w = np.zeros((128, 128), dtype=np.float32)
    for j in range(127):
        w[j, j] = -1.0
        w[j + 1, j] = 1.0
    wfix = np.zeros((128, 128), dtype=np.float32)
    wfix[0, 126] = 1.0
    return w, wfix


def _per_core_inputs(states_b, mask_b, rp_b, w, wfix, modes, offs,
                     total_cols):
    # weighted-sum coefficients: mw[p, i] = mask[t+2] * weight[t], t = 127i+p
    t = np.arange(L - 2, dtype=np.float64)
    dist = np.maximum(float(rp_b) - t - 2.0, 0.0)
    weight = np.where(dist < 5, 2.0 + (5.0 - dist) * 0.5, 1.0).astype(np.float32)
    mwvec = (mask_b[2:L] * weight).astype(np.float32)  # [L-2]
    vals = np.zeros(NT * STRIDE, dtype=np.float32)
    vals[: L - 2] = mwvec
    mw = np.zeros((128, NT), dtype=np.float32)
    mw[:STRIDE, :] = vals.reshape(NT, STRIDE).T

    mt = mask_b.astype(np.float32).copy()
    mt[0:2] = 0.0
    maskt = mt.reshape(128, 32)

    # window-major fp8 staging; 'f' windows also stage the one-token-
    # shifted copy so the DVE subtract is partition-aligned
    sf8 = states_b.astype(ml_dtypes.float8_e4m3)
    s2 = np.zeros((128, total_cols), dtype=ml_dtypes.float8_e4m3)
    for i, m in enumerate(modes):
        r0 = STRIDE * i
        rows = min(128, L - r0)
        o = offs[i]
        s2[:rows, o : o + D] = sf8[r0 : r0 + rows]
        if m == "f":
            rows2 = min(128, L - r0 - 1)
            s2[:rows2, o + D : o + 2 * D] = sf8[r0 + 1 : r0 + 1 + rows2]

    return {
        "s2": s2,
        "wshift": w.astype(np.float16),
        "wfix": wfix.astype(np.float16),
        "wf8": w.astype(ml_dtypes.float8_e4m3),
        "mw": mw,
        "maskt": np.ascontiguousarray(maskt),
    }


def _get_nc(split_mode=DEFAULT_SPLIT):
    key = ("nc", split_mode)
    if key not in _CACHE:
        _CACHE[key] = _build_nc(split_mode)
    return _CACHE[key]


def _run(states, reasoning_mask, result_token_positions, trace=False,
         split_mode=DEFAULT_SPLIT):
    from concourse.bass_utils import run_bass_kernel_spmd

    states = np.asarray(states, dtype=np.float32)
    mask = np.asarray(reasoning_mask, dtype=np.float32)
    rp = np.asarray(result_token_positions)

    modes = _parse_modes(split_mode)
    offs, widths, total_cols, groups = _layout(modes)
    w, wfix = _host_consts()
    in_maps = [
        _per_core_inputs(
            states[b], mask[b], rp[b], w, wfix, modes, offs, total_cols
        )
        for b in range(N_CORES)
    ]
    nc = _get_nc(split_mode)
    res = run_bass_kernel_spmd(
        nc, in_maps, core_ids=list(range(N_CORES)), trace=trace
    )
    partials = np.stack([res.results[i]["out"][0] for i in range(N_CORES)])  # [8, 2]
    s = partials[:, 0].astype(np.float64).sum()
    m = partials[:, 1].astype(np.float64).sum()
    value = np.float32(s / (m + 1e-9))
    return value, res


def kernel(states, reasoning_mask, result_token_positions):
    value, _ = _run(states, reasoning_mask, result_token_positions)
    return np.asarray(value, dtype=np.float32)
